# revision 7
# baseline (speedup 1.0000x reference)
"""Trainium2 Bass kernel for 4-bit-quantized Linear: y = x @ dequant(Wq4).T + bias.

v3: single 3-bank PSUM tile per token-tile (one [128,1408] DVE drain + one
store instead of three each), tile-0 x loads hoisted ahead of the weight
stream, w16 spread over scalar/sync/gpsimd queues to cut the per-call
weight-load race. Packed single-blob input (per-arg dispatch cost dominates
the per-call overhead on the PJRT/axon path).
"""
import numpy as np
import ml_dtypes

import concourse.bass as bass
import concourse.bacc as bacc
import concourse.mybir as mybir
import concourse.tile as tile
from concourse.bass_utils import run_bass_kernel_spmd

F16, F32, F8 = mybir.dt.float16, mybir.dt.float32, mybir.dt.float8e4

TOKENS, IN, OUT = 4096, 4096, 11008
GROUP, BLOCKS, HALF = 16, 256, 8
N_CORES = 8
O_C = 1408
KT = IN // 128
TC = 256
O_CHUNKS = [(0, 512), (512, 512), (1024, 384)]
N8 = 8
N16 = KT - N8


def _blob_layout(n8):
    n16 = KT - n8
    n_tc = TOKENS // TC
    x16_seg = n16 * TC * 2
    x8_seg = n8 * TC
    off = {}
    cur = 0
    off["x16"] = cur; cur += n_tc * x16_seg
    off["w16"] = cur; cur += n16 * O_C * 2
    off["x8"] = cur; cur += n_tc * x8_seg
    off["w8"] = cur; cur += n8 * O_C
    off["bias"] = cur; cur += O_C * 4
    return off, cur, x16_seg, x8_seg


def build_bass(n8=N8, reps=1):
    n16 = KT - n8
    pairs = n8 // 2
    n_tc = TOKENS // TC
    tl_per_tc = TC // 128
    off, total, x16_seg, x8_seg = _blob_layout(n8)

    nc = bacc.Bacc("TRN2", target_bir_lowering=False, debug=False)

    blob = nc.dram_tensor("blob", [128, total], mybir.dt.uint8,
                          kind="ExternalInput")
    y_d = nc.dram_tensor("y", [TOKENS, O_C], F16, kind="ExternalOutput")

    def x16_view(tci):
        a = off["x16"] + tci * x16_seg
        return blob[:, a:a + x16_seg].bitcast(F16).rearrange(
            "p (s t) -> p s t", s=n16)

    def x8_view(tci):
        a = off["x8"] + tci * x8_seg
        return blob[:, a:a + x8_seg].bitcast(F8).rearrange(
            "p (s t) -> p s t", s=n8)

    w16_v = blob[:, off["w16"]:off["w16"] + n16 * O_C * 2].bitcast(
        F16).rearrange("p (s o) -> p s o", s=n16)
    if n8:
        w8_v = blob[:, off["w8"]:off["w8"] + n8 * O_C].bitcast(
            F8).rearrange("p (s o) -> p s o", s=n8)
    bias_v = blob[:, off["bias"]:off["bias"] + O_C * 4].bitcast(F32)

    with tile.TileContext(nc) as tc:
        with (
            tc.tile_pool(name="const", bufs=1) as cst,
            tc.tile_pool(name="wp", bufs=1) as wp,
            tc.tile_pool(name="xp", bufs=2) as xp,
            tc.tile_pool(name="yp", bufs=2) as yp,
            tc.tile_pool(name="psm", bufs=2, space=bass.MemorySpace.PSUM) as psm,
        ):
            xh = n16 // 2

            def x_load(tci):
                tiles = []
                if n8:
                    x8t = xp.tile([128, n8, TC], F8, tag="x8")
                    nc.gpsimd.dma_start(x8t[:], x8_view(tci))
                    tiles.append(x8t)
                else:
                    tiles.append(None)
                x16t = xp.tile([128, n16, TC], F16, tag="x16")
                v = x16_view(tci)
                nc.gpsimd.dma_start(x16t[:, :xh], v[:, :xh])
                nc.scalar.dma_start(x16t[:, xh:], v[:, xh:])
                tiles.append(x16t)
                return tiles

            # tile-0 x loads first: the pair matmuls only need x8+w8, so the
            # PE starts within ~the w8 transfer; w16 streams behind on all
            # three queues and the fp16 k-loop chases it slab by slab.
            pre = x_load(0)

            if n8:
                w8_sb = wp.tile([128, n8, O_C], F8, tag="w8")
                for p in range(n8 // 2):
                    nc.scalar.dma_start(
                        w8_sb[:, 2 * p:2 * p + 2], w8_v[:, 2 * p:2 * p + 2])
            w16_sb = wp.tile([128, n16, O_C], F16, tag="w16")
            engs = [nc.scalar, nc.sync, nc.gpsimd]
            for s in range(n16):
                engs[s % 3].dma_start(w16_sb[:, s], w16_v[:, s])
            bias_sb = cst.tile([128, O_C], F32, tag="bias")
            nc.sync.dma_start(bias_sb[:], bias_v)

            for rep in range(reps):
                for tci in range(n_tc):
                    if rep == 0 and tci == 0:
                        x8t, x16t = pre
                    else:
                        x8t, x16t = x_load(tci)
                    for tl in range(tl_per_tc):
                        y_sb = yp.tile([128, O_C], F16, tag="y")
                        ps = psm.tile([128, O_C], F32, tag="ps")
                        ts = slice(tl * 128, (tl + 1) * 128)
                        for p in range(pairs):
                            for (o_off, o_w) in O_CHUNKS:
                                nc.tensor.matmul(
                                    ps[:, o_off:o_off + o_w],
                                    x8t[:, 2 * p:2 * p + 2, ts],
                                    w8_sb[:, 2 * p:2 * p + 2, o_off:o_off + o_w],
                                    start=(p == 0), stop=False,
                                    perf_mode=mybir.MatmulPerfMode.DoubleRow)
                        row = y_d[tci * TC + tl * 128:
                                  tci * TC + (tl + 1) * 128, :]
                        last_tile = (rep == reps - 1 and tci == n_tc - 1
                                     and tl == tl_per_tc - 1)
                        if last_tile:
                            # chunk-outer for the very last tile: each chunk
                            # closes and drains while later chunks' matmuls
                            # still run, shrinking the serial tail
                            for (o_off, o_w) in O_CHUNKS:
                                for s in range(n16):
                                    nc.tensor.matmul(
                                        ps[:, o_off:o_off + o_w],
                                        x16t[:, s, ts],
                                        w16_sb[:, s, o_off:o_off + o_w],
                                        start=(pairs == 0 and s == 0),
                                        stop=(s == n16 - 1))
                                nc.vector.tensor_tensor(
                                    y_sb[:, o_off:o_off + o_w],
                                    ps[:, o_off:o_off + o_w],
                                    bias_sb[:, o_off:o_off + o_w],
                                    mybir.AluOpType.add)
                                nc.gpsimd.dma_start(
                                    row[:, o_off:o_off + o_w],
                                    y_sb[:, o_off:o_off + o_w])
                        else:
                            for s in range(n16):
                                for (o_off, o_w) in O_CHUNKS:
                                    nc.tensor.matmul(
                                        ps[:, o_off:o_off + o_w],
                                        x16t[:, s, ts],
                                        w16_sb[:, s, o_off:o_off + o_w],
                                        start=(pairs == 0 and s == 0),
                                        stop=(s == n16 - 1))
                            # single full-width drain + store (3-bank PSUM
                            # tile): 1 DVE op + 1 DMA instead of 3+3
                            nc.vector.tensor_tensor(
                                y_sb[:], ps[:], bias_sb[:],
                                mybir.AluOpType.add)
                            nc.gpsimd.dma_start(row[:], y_sb[:])
    nc.compile()
    return nc


def _dequant_np(weight_q4, weight_norm):
    low = weight_q4 & 15
    high = (weight_q4 >> 4) & 15
    q8 = np.stack((low, high), axis=-1).reshape(OUT, BLOCKS, GROUP)
    q8 = q8.astype(np.float32) / 15.0
    norms = weight_norm.astype(np.float32)
    return (q8 * 2.0 * norms - norms).reshape(OUT, IN)


def _prep_host_inputs(x, weight_q4, weight_norm, bias, n8=N8):
    n16 = KT - n8
    n_tc = TOKENS // TC
    off, total, x16_seg, x8_seg = _blob_layout(n8)

    xT = np.ascontiguousarray(x.T)
    xs = xT.reshape(KT, 128, n_tc, TC).transpose(2, 1, 0, 3)
    x16 = np.ascontiguousarray(
        xs[:, :, n8:].astype(np.float16).transpose(1, 0, 2, 3).reshape(128, -1))
    x16_u8 = x16.view(np.uint8)
    if n8:
        x8 = np.ascontiguousarray(np.asarray(
            xs[:, :, :n8], dtype=ml_dtypes.float8_e4m3
        ).transpose(1, 0, 2, 3).reshape(128, -1))
        x8_u8 = x8.view(np.uint8)

    W = _dequant_np(weight_q4, weight_norm)
    o_pad = N_CORES * O_C
    Wp = np.zeros((o_pad, IN), np.float32)
    Wp[:OUT] = W
    bs = np.zeros((o_pad,), np.float32)
    bs[:OUT] = bias

    in_maps = []
    for c in range(N_CORES):
        Wc = Wp[c * O_C:(c + 1) * O_C]
        WcT = Wc.T.reshape(KT, 128, O_C)
        w16 = np.ascontiguousarray(
            WcT[n8:].transpose(1, 0, 2).astype(np.float16)).reshape(128, -1)
        bias_rep = np.ascontiguousarray(np.broadcast_to(
            bs[c * O_C:(c + 1) * O_C][None, :], (128, O_C))).astype(np.float32)
        segs = [x16_u8, w16.view(np.uint8)]
        if n8:
            w8 = np.ascontiguousarray(np.asarray(
                WcT[:n8].transpose(1, 0, 2), dtype=ml_dtypes.float8_e4m3)
            ).reshape(128, -1)
            segs += [x8_u8, w8.view(np.uint8)]
        segs.append(bias_rep.view(np.uint8))
        blob = np.ascontiguousarray(np.hstack(segs))
        assert blob.shape == (128, total), (blob.shape, total)
        in_maps.append({"blob": blob})
    return in_maps


_CACHE = {}


def _make_runner(nc):
    """Persistent jitted SPMD runner (mirrors bass2jax.run_bass_via_pjrt's
    multi-core branch) so repeated kernel() calls reuse one executable."""
    import jax
    from jax.sharding import Mesh, PartitionSpec, NamedSharding
    try:
        from jax.experimental.shard_map import shard_map
    except ImportError:
        from jax.shard_map import shard_map
    from concourse.bass2jax import (
        _bass_exec_p, partition_id_tensor, install_neuronx_cc_hook)

    install_neuronx_cc_hook()
    partition_name = (
        nc.partition_id_tensor.name if nc.partition_id_tensor else None)
    in_names, out_names, out_avals, zero_shapes = [], [], [], []
    for alloc in nc.m.functions[0].allocations:
        if not isinstance(alloc, mybir.MemoryLocationSet):
            continue
        name = alloc.memorylocations[0].name
        if alloc.kind == "ExternalInput":
            if name != partition_name:
                in_names.append(name)
        elif alloc.kind == "ExternalOutput":
            out_names.append(name)
            shape = tuple(alloc.tensor_shape)
            dtype = mybir.dt.np(alloc.dtype)
            out_avals.append(jax.core.ShapedArray(shape, dtype))
            zero_shapes.append((shape, dtype))
    n_params = len(in_names)
    n_outs = len(out_avals)
    all_in = list(in_names) + list(out_names)
    if partition_name is not None:
        all_in.append(partition_name)
    donate = tuple(range(n_params, n_params + n_outs))

    def _body(*args):
        operands = list(args)
        if partition_name is not None:
            operands.append(partition_id_tensor())
        outs = _bass_exec_p.bind(
            *operands,
            out_avals=tuple(out_avals),
            in_names=tuple(all_in),
            out_names=tuple(out_names),
            lowering_input_output_aliases=(),
            sim_require_finite=True,
            sim_require_nnan=True,
            nc=nc,
        )
        return tuple(outs)

    devices = jax.devices()[:N_CORES]
    mesh = Mesh(np.asarray(devices), ("core",))
    in_specs = (PartitionSpec("core"),) * (n_params + n_outs)
    out_specs = (PartitionSpec("core"),) * n_outs
    sharded = jax.jit(
        shard_map(_body, mesh=mesh, in_specs=in_specs,
                  out_specs=out_specs, check_rep=False),
        donate_argnums=donate, keep_unused=True)

    def run(in_maps):
        concat_in = [
            np.concatenate([np.asarray(in_maps[c][n]) for c in range(N_CORES)],
                           axis=0)
            for n in in_names
        ]
        zeros = [np.zeros((N_CORES * s[0], *s[1:]), d) for s, d in zero_shapes]
        out_arrs = sharded(*concat_in, *zeros)
        return [
            {name: np.asarray(out_arrs[i]).reshape(
                N_CORES, *out_avals[i].shape)[c]
             for i, name in enumerate(out_names)}
            for c in range(N_CORES)
        ]
    return run


def _run(in_maps):
    if "nc" not in _CACHE:
        _CACHE["nc"] = build_bass()
    nc = _CACHE["nc"]
    if "runner" not in _CACHE:
        try:
            _CACHE["runner"] = _make_runner(nc)
        except Exception:
            _CACHE["runner"] = None
    if _CACHE["runner"] is not None:
        try:
            return _CACHE["runner"](in_maps)
        except Exception:
            _CACHE["runner"] = None
    return run_bass_kernel_spmd(nc, in_maps, list(range(N_CORES))).results


def kernel(x, weight_q4, weight_norm, bias):
    in_maps = _prep_host_inputs(
        np.asarray(x), np.asarray(weight_q4),
        np.asarray(weight_norm), np.asarray(bias))
    results = _run(in_maps)
    outs = [results[c]["y"] for c in range(N_CORES)]
    y = np.concatenate(outs, axis=1)[:, :OUT]
    return np.ascontiguousarray(y.astype(np.float32))
